# revision 6
# baseline (speedup 1.0000x reference)
"""LSTM cell forward (nn_CellLSTM) on 8 trn2 NeuronCores.

Math (per reference):
    gates[g] = x @ ih4[g] + h_0 @ hh4[g] + ib4[g] + hb4[g]   for g in I,F,G,O
    c_1 = c_0 * sigmoid(F) + sigmoid(I) * tanh(G)
    h_1 = sigmoid(O) + tanh(c_1)
Outputs: (h_1, c_1, I_g, F_g, G_g, O_g), each [B, H].

Sharding: pure data parallel over the batch axis; each of the 8 cores gets a
contiguous slab of B/8 = 16384 rows; ih/hh/ib/hb replicated. No collectives.

Per-core kernel layout (batch-major):
  - supertile = 1024 batch rows mapped as b = b0 + 8*p + r (p=partition,
    r in [0,8)); every HBM transfer is 128 descriptors x 4KB contiguous.
  - per 128-row subtile: PE-transpose x,h -> PSUM -> SBUF, then
    gates_psum[128,512] = xT.T @ Wih[128,512] + hT.T @ Whh + ones.T @ (ib+hb)
    (weights pre-concatenated over the 4 gates in the free dim; bias added
    via a rank-1 K=1 matmul; float32r streaming for 1 cycle/row).
  - ScalarE: sigmoid/tanh of the gate slices straight from PSUM;
    VectorE: raw-gate PSUM->SBUF copy and the c_1 combine;
    GpSimd: the final h_1 add.
"""

import numpy as np

import concourse.bass as bass
import concourse.bacc as bacc
import concourse.mybir as mybir
import concourse.tile as tile
from concourse import bass_utils
from concourse.masks import make_identity

N_CORES = 8
B_FULL = 131072
H = 128
ROWS_PER_CORE = B_FULL // N_CORES

SUPER = 1024          # batch rows per supertile
RPP = SUPER // 128    # rows per partition = subtiles per supertile

F32 = mybir.dt.float32
F32R = mybir.dt.float32r
AFT = mybir.ActivationFunctionType

USE_F32R = True

OUT_NAMES = ("h_1", "c_1", "I_g", "F_g", "G_g", "O_g")


def build_nc(rows=ROWS_PER_CORE, super_rows=SUPER):
    rpp = super_rows // 128
    assert rows % super_rows == 0
    n_super = rows // super_rows

    nc = bacc.Bacc("TRN2", target_bir_lowering=False)

    x = nc.dram_tensor("x", [rows, H], F32, kind="ExternalInput")
    h0 = nc.dram_tensor("h_0", [rows, H], F32, kind="ExternalInput")
    c0 = nc.dram_tensor("c_0", [rows, H], F32, kind="ExternalInput")
    ih = nc.dram_tensor("ih", [4 * H, H], F32, kind="ExternalInput")
    hh = nc.dram_tensor("hh", [4 * H, H], F32, kind="ExternalInput")
    ib = nc.dram_tensor("ib", [4 * H], F32, kind="ExternalInput")
    hb = nc.dram_tensor("hb", [4 * H], F32, kind="ExternalInput")
    outs = {
        name: nc.dram_tensor(name, [rows, H], F32, kind="ExternalOutput")
        for name in OUT_NAMES
    }

    MMDT = F32R if USE_F32R else F32

    # HBM views: [n_super, 128 partitions, rpp*H] with b = s*super + p*rpp + r
    def view(t):
        return t.ap().rearrange("(s p r) i -> s p (r i)", p=128, r=rpp)

    xv, hv, cv = view(x), view(h0), view(c0)
    ov = {name: view(t) for name, t in outs.items()}

    with tile.TileContext(nc) as tc:
        with (
            tc.tile_pool(name="const", bufs=1) as cpool,
            tc.tile_pool(name="io", bufs=3) as iop,
            tc.tile_pool(name="trp", bufs=2, space="PSUM") as trp,
            tc.tile_pool(name="pgp", bufs=3, space="PSUM") as pgp,
            tc.tile_pool(name="sbt", bufs=3) as sbt,
            tc.tile_pool(name="gsb", bufs=2) as gsb,
            tc.tile_pool(name="actp", bufs=2) as actp,
            tc.tile_pool(name="resp", bufs=2) as resp,
        ):
            ident = cpool.tile([128, 128], F32)
            make_identity(nc, ident[:])

            # Wih[h, g*128+i] = ih[g*128+h, i]; same for Whh.
            wih_raw = cpool.tile([128, 4 * H], F32)
            whh_raw = cpool.tile([128, 4 * H], F32)
            for g in range(4):
                gs = slice(g * H, (g + 1) * H)
                nc.sync.dma_start(wih_raw[:, gs], ih.ap()[gs, :])
                nc.sync.dma_start(whh_raw[:, gs], hh.ap()[gs, :])
            # round once to the matmul streaming dtype (f32r producers must
            # explicitly round; these are one-time const-setup copies)
            wih = cpool.tile([128, 4 * H], MMDT)
            whh = cpool.tile([128, 4 * H], MMDT)
            nc.vector.tensor_copy(wih[:], wih_raw[:])
            nc.vector.tensor_copy(whh[:], whh_raw[:])

            ones_raw = cpool.tile([1, 128], F32)
            nc.vector.memset(ones_raw[:], 1.0)
            ones = cpool.tile([1, 128], MMDT)
            nc.vector.tensor_copy(ones[:], ones_raw[:])
            bib = cpool.tile([1, 4 * H], F32)
            bhb = cpool.tile([1, 4 * H], F32)
            nc.sync.dma_start(bib[:], ib.ap()[None, :])
            nc.sync.dma_start(bhb[:], hb.ap()[None, :])
            bsum_raw = cpool.tile([1, 4 * H], F32)
            nc.vector.tensor_add(bsum_raw[:], bib[:], bhb[:])
            bsum = cpool.tile([1, 4 * H], MMDT)
            nc.vector.tensor_copy(bsum[:], bsum_raw[:])

            for s in range(n_super):
                x_in = iop.tile([128, super_rows], F32)
                nc.sync.dma_start(x_in[:], xv[s])
                h_in = iop.tile([128, super_rows], F32)
                nc.sync.dma_start(h_in[:], hv[s])
                c_in = iop.tile([128, super_rows], F32)
                nc.sync.dma_start(c_in[:], cv[s])

                gates = gsb.tile([128, rpp * 512], F32)
                sigI = actp.tile([128, super_rows], F32)
                sigF = actp.tile([128, super_rows], F32)
                tanG = actp.tile([128, super_rows], F32)
                sigO = actp.tile([128, super_rows], F32)

                for r in range(rpp):
                    rs = slice(r * 128, (r + 1) * 128)
                    trx = trp.tile([128, 128], F32)
                    nc.tensor.transpose(trx[:], x_in[:, rs], ident[:])
                    trh = trp.tile([128, 128], F32)
                    nc.tensor.transpose(trh[:], h_in[:, rs], ident[:])
                    xT = sbt.tile([128, 128], MMDT)
                    nc.scalar.copy(xT[:], trx[:])
                    hT = sbt.tile([128, 128], MMDT)
                    nc.scalar.copy(hT[:], trh[:])

                    pg = pgp.tile([128, 512], F32)
                    nc.tensor.matmul(pg[:], xT[:], wih[:], start=True, stop=False)
                    nc.tensor.matmul(pg[:], hT[:], whh[:], start=False, stop=False)
                    nc.tensor.matmul(pg[:], ones[:], bsum[:], start=False, stop=True)

                    # raw (pre-activation) gates out
                    nc.vector.tensor_copy(gates[:, r * 512 : (r + 1) * 512], pg[:])
                    nc.scalar.activation(sigI[:, rs], pg[:, 0:128], AFT.Sigmoid)
                    nc.scalar.activation(sigF[:, rs], pg[:, 128:256], AFT.Sigmoid)
                    nc.scalar.activation(tanG[:, rs], pg[:, 256:384], AFT.Tanh)
                    nc.scalar.activation(sigO[:, rs], pg[:, 384:512], AFT.Sigmoid)

                t1 = resp.tile([128, super_rows], F32)
                nc.vector.tensor_mul(t1[:], c_in[:], sigF[:])
                t2 = resp.tile([128, super_rows], F32)
                nc.vector.tensor_mul(t2[:], sigI[:], tanG[:])
                c1t = resp.tile([128, super_rows], F32)
                nc.vector.tensor_add(c1t[:], t1[:], t2[:])
                th1 = actp.tile([128, super_rows], F32)
                nc.scalar.activation(th1[:], c1t[:], AFT.Tanh)
                h1t = resp.tile([128, super_rows], F32)
                nc.gpsimd.tensor_add(h1t[:], sigO[:], th1[:])

                nc.sync.dma_start(ov["h_1"][s], h1t[:])
                nc.sync.dma_start(ov["c_1"][s], c1t[:])
                gr = gates[:].rearrange("p (r gi) -> p r gi", gi=512)
                for g, name in enumerate(("I_g", "F_g", "G_g", "O_g")):
                    nc.sync.dma_start(ov[name][s], gr[:, :, g * H : (g + 1) * H])

    nc.compile()
    return nc


_NC_CACHE = {}


def _get_nc(rows=ROWS_PER_CORE):
    if rows not in _NC_CACHE:
        _NC_CACHE[rows] = build_nc(rows)
    return _NC_CACHE[rows]


def run_sharded(x, h_0, c_0, ih, hh, ib, hb, **spmd_kwargs):
    x = np.asarray(x, dtype=np.float32)
    h_0 = np.asarray(h_0, dtype=np.float32)
    c_0 = np.asarray(c_0, dtype=np.float32)
    ih = np.ascontiguousarray(np.asarray(ih, dtype=np.float32))
    hh = np.ascontiguousarray(np.asarray(hh, dtype=np.float32))
    ib = np.ascontiguousarray(np.asarray(ib, dtype=np.float32))
    hb = np.ascontiguousarray(np.asarray(hb, dtype=np.float32))

    nc = _get_nc()
    in_maps = []
    for i in range(N_CORES):
        sl = slice(i * ROWS_PER_CORE, (i + 1) * ROWS_PER_CORE)
        in_maps.append(
            dict(
                x=np.ascontiguousarray(x[sl]),
                h_0=np.ascontiguousarray(h_0[sl]),
                c_0=np.ascontiguousarray(c_0[sl]),
                ih=ih,
                hh=hh,
                ib=ib,
                hb=hb,
            )
        )
    res = bass_utils.run_bass_kernel_spmd(
        nc, in_maps, core_ids=list(range(N_CORES)), **spmd_kwargs
    )
    outs = res.results
    full = tuple(
        np.concatenate([outs[i][name] for i in range(N_CORES)], axis=0)
        for name in OUT_NAMES
    )
    return full, res


def kernel(x, h_0, c_0, ih, hh, ib, hb):
    full, _ = run_sharded(x, h_0, c_0, ih, hh, ib, hb)
    return full


# revision 12
# speedup vs baseline: 1.4286x; 1.4286x over previous
"""LSTM cell forward (nn_CellLSTM) on 8 trn2 NeuronCores.

Math (per reference):
    gates[g] = x @ ih4[g] + h_0 @ hh4[g] + ib4[g] + hb4[g]   for g in I,F,G,O
    c_1 = c_0 * sigmoid(F) + sigmoid(I) * tanh(G)
    h_1 = sigmoid(O) + tanh(c_1)
Outputs: (h_1, c_1, I_g, F_g, G_g, O_g), each [B, H].

Sharding: pure data parallel over the batch axis; each of the 8 cores gets a
contiguous slab of B/8 = 16384 rows; ih/hh/ib/hb replicated. No collectives.

Per-core kernel layout (batch-major):
  - supertile = 1024 batch rows mapped as b = b0 + 8*p + r (p=partition,
    r in [0,8)); every HBM transfer is 128 descriptors x 4KB contiguous.
  - per 128-row subtile: PE-transpose x,h -> PSUM -> SBUF, then
    gates_psum[128,512] = xT.T @ Wih[128,512] + hT.T @ Whh + ones.T @ (ib+hb)
    (weights pre-concatenated over the 4 gates in the free dim; bias added
    via a rank-1 K=1 matmul; float32r streaming for 1 cycle/row).
  - ScalarE: sigmoid/tanh of the gate slices straight from PSUM;
    VectorE: raw-gate PSUM->SBUF copy and the c_1 combine;
    GpSimd: the final h_1 add.
"""

import numpy as np

import concourse.bass as bass
import concourse.bacc as bacc
import concourse.mybir as mybir
import concourse.tile as tile
from concourse import bass_utils
from concourse.masks import make_identity

N_CORES = 8
B_FULL = 131072
H = 128
ROWS_PER_CORE = B_FULL // N_CORES

SUPER = 1024          # batch rows per supertile
RPP = SUPER // 128    # rows per partition = subtiles per supertile

F32 = mybir.dt.float32
F32R = mybir.dt.float32r
AFT = mybir.ActivationFunctionType

USE_F32R = True

OUT_NAMES = ("h_1", "c_1", "I_g", "F_g", "G_g", "O_g")


def build_nc(rows=ROWS_PER_CORE, super_rows=SUPER, repeat=1):
    rpp = super_rows // 128
    assert rows % super_rows == 0
    n_super = rows // super_rows

    nc = bacc.Bacc("TRN2", target_bir_lowering=False)

    x = nc.dram_tensor("x", [rows, H], F32, kind="ExternalInput")
    h0 = nc.dram_tensor("h_0", [rows, H], F32, kind="ExternalInput")
    c0 = nc.dram_tensor("c_0", [rows, H], F32, kind="ExternalInput")
    ih = nc.dram_tensor("ih", [4 * H, H], F32, kind="ExternalInput")
    hh = nc.dram_tensor("hh", [4 * H, H], F32, kind="ExternalInput")
    ib = nc.dram_tensor("ib", [4 * H], F32, kind="ExternalInput")
    hb = nc.dram_tensor("hb", [4 * H], F32, kind="ExternalInput")
    outs = {
        name: nc.dram_tensor(name, [rows, H], F32, kind="ExternalOutput")
        for name in OUT_NAMES
    }

    MMDT = F32R if USE_F32R else F32

    # HBM views: [n_super, 128 partitions, rpp*H] with b = s*super + p*rpp + r
    def view(t):
        return t.ap().rearrange("(s p r) i -> s p (r i)", p=128, r=rpp)

    xv, hv, cv = view(x), view(h0), view(c0)
    ov = {name: view(t) for name, t in outs.items()}

    with tile.TileContext(nc) as tc:
        with (
            tc.tile_pool(name="const", bufs=1) as cpool,
            tc.tile_pool(name="io", bufs=4) as iop,
            tc.tile_pool(name="trp", bufs=3, space="PSUM") as trp,
            tc.tile_pool(name="pgp", bufs=4, space="PSUM") as pgp,
            tc.tile_pool(name="sbt", bufs=3) as sbt,
            tc.tile_pool(name="gsb", bufs=3) as gsb,
            tc.tile_pool(name="actp", bufs=2) as actp,
            tc.tile_pool(name="resp", bufs=2) as resp,
        ):
            ident = cpool.tile([128, 128], F32)
            make_identity(nc, ident[:])

            # Wih[h, g*128+i] = ih[g*128+h, i]; same for Whh.
            wih_raw = cpool.tile([128, 4 * H], F32)
            whh_raw = cpool.tile([128, 4 * H], F32)
            for g in range(4):
                gs = slice(g * H, (g + 1) * H)
                nc.sync.dma_start(wih_raw[:, gs], ih.ap()[gs, :])
                nc.sync.dma_start(whh_raw[:, gs], hh.ap()[gs, :])
            # round once to the matmul streaming dtype (f32r producers must
            # explicitly round; these are one-time const-setup copies)
            wih = cpool.tile([128, 4 * H], MMDT)
            whh = cpool.tile([128, 4 * H], MMDT)
            nc.vector.tensor_copy(wih[:], wih_raw[:])
            nc.vector.tensor_copy(whh[:], whh_raw[:])

            ones_raw = cpool.tile([1, 128], F32)
            nc.vector.memset(ones_raw[:], 1.0)
            ones = cpool.tile([1, 128], MMDT)
            nc.vector.tensor_copy(ones[:], ones_raw[:])
            bib = cpool.tile([1, 4 * H], F32)
            bhb = cpool.tile([1, 4 * H], F32)
            nc.sync.dma_start(bib[:], ib.ap()[None, :])
            nc.sync.dma_start(bhb[:], hb.ap()[None, :])
            bsum_raw = cpool.tile([1, 4 * H], F32)
            nc.vector.tensor_add(bsum_raw[:], bib[:], bhb[:])
            bsum = cpool.tile([1, 4 * H], MMDT)
            nc.vector.tensor_copy(bsum[:], bsum_raw[:])

            for s in [s for _ in range(repeat) for s in range(n_super)]:
                x_in = iop.tile([128, super_rows], F32)
                nc.sync.dma_start(x_in[:], xv[s])
                h_in = iop.tile([128, super_rows], F32)
                nc.sync.dma_start(h_in[:], hv[s])
                c_in = iop.tile([128, super_rows], F32)
                nc.sync.dma_start(c_in[:], cv[s])

                gates = gsb.tile([128, rpp * 512], F32)
                sigI = actp.tile([128, super_rows], F32)
                sigF = actp.tile([128, super_rows], F32)
                tanG = actp.tile([128, super_rows], F32)
                sigO = actp.tile([128, super_rows], F32)

                for r in range(rpp):
                    rs = slice(r * 128, (r + 1) * 128)
                    # both transposes into ONE psum bank as one accumulation
                    # group (disjoint slices) -> a single pair-copy to SBUF
                    tr = trp.tile([128, 512], F32)
                    nc.tensor.matmul(
                        tr[:, 0:128], x_in[:, rs], ident[:],
                        is_transpose=True, start=True, stop=False,
                    )
                    nc.tensor.matmul(
                        tr[:, 128:256], h_in[:, rs], ident[:],
                        is_transpose=True, start=False, stop=True,
                    )
                    xhT = sbt.tile([128, 256], MMDT)
                    nc.scalar.copy(xhT[:], tr[:, 0:256])

                    pg = pgp.tile([128, 512], F32)
                    nc.tensor.matmul(pg[:], xhT[:, 0:128], wih[:], start=True, stop=False)
                    nc.tensor.matmul(pg[:], xhT[:, 128:256], whh[:], start=False, stop=False)
                    nc.tensor.matmul(pg[:], ones[:], bsum[:], start=False, stop=True)

                    # raw (pre-activation) gates out
                    nc.vector.tensor_copy(gates[:, r * 512 : (r + 1) * 512], pg[:])

                # gate activations: one strided op per gate over the whole
                # supertile, reading the SBUF raw-gates copy
                gr3 = gates[:].rearrange("p (r gi) -> p r gi", gi=512)
                for g, dst in enumerate((sigI, sigF, tanG, sigO)):
                    func = AFT.Tanh if g == 2 else AFT.Sigmoid
                    src = gr3[:, :, g * 128 : (g + 1) * 128]
                    d3 = dst[:].rearrange("p (r i) -> p r i", i=128)
                    nc.scalar.activation(d3, src, func)

                t1 = resp.tile([128, super_rows], F32)
                nc.vector.tensor_mul(t1[:], c_in[:], sigF[:])
                t2 = resp.tile([128, super_rows], F32)
                nc.vector.tensor_mul(t2[:], sigI[:], tanG[:])
                c1t = resp.tile([128, super_rows], F32)
                nc.vector.tensor_add(c1t[:], t1[:], t2[:])
                th1 = actp.tile([128, super_rows], F32)
                nc.scalar.activation(th1[:], c1t[:], AFT.Tanh)
                h1t = resp.tile([128, super_rows], F32)
                nc.gpsimd.tensor_add(h1t[:], sigO[:], th1[:])

                nc.sync.dma_start(ov["h_1"][s], h1t[:])
                nc.sync.dma_start(ov["c_1"][s], c1t[:])
                gr = gates[:].rearrange("p (r gi) -> p r gi", gi=512)
                for g, name in enumerate(("I_g", "F_g", "G_g", "O_g")):
                    nc.sync.dma_start(ov[name][s], gr[:, :, g * H : (g + 1) * H])

    nc.compile()
    return nc


_NC_CACHE = {}


def _get_nc(rows=ROWS_PER_CORE):
    if rows not in _NC_CACHE:
        _NC_CACHE[rows] = build_nc(rows)
    return _NC_CACHE[rows]


def run_sharded(x, h_0, c_0, ih, hh, ib, hb, **spmd_kwargs):
    x = np.asarray(x, dtype=np.float32)
    h_0 = np.asarray(h_0, dtype=np.float32)
    c_0 = np.asarray(c_0, dtype=np.float32)
    ih = np.ascontiguousarray(np.asarray(ih, dtype=np.float32))
    hh = np.ascontiguousarray(np.asarray(hh, dtype=np.float32))
    ib = np.ascontiguousarray(np.asarray(ib, dtype=np.float32))
    hb = np.ascontiguousarray(np.asarray(hb, dtype=np.float32))

    nc = _get_nc()
    in_maps = []
    for i in range(N_CORES):
        sl = slice(i * ROWS_PER_CORE, (i + 1) * ROWS_PER_CORE)
        in_maps.append(
            dict(
                x=np.ascontiguousarray(x[sl]),
                h_0=np.ascontiguousarray(h_0[sl]),
                c_0=np.ascontiguousarray(c_0[sl]),
                ih=ih,
                hh=hh,
                ib=ib,
                hb=hb,
            )
        )
    res = bass_utils.run_bass_kernel_spmd(
        nc, in_maps, core_ids=list(range(N_CORES)), **spmd_kwargs
    )
    outs = res.results
    full = tuple(
        np.concatenate([outs[i][name] for i in range(N_CORES)], axis=0)
        for name in OUT_NAMES
    )
    return full, res


def kernel(x, h_0, c_0, ih, hh, ib, hb):
    full, _ = run_sharded(x, h_0, c_0, ih, hh, ib, hb)
    return full


# revision 14
# speedup vs baseline: 5.5487x; 3.8840x over previous
"""LSTM cell forward (nn_CellLSTM) on 8 trn2 NeuronCores.

Math (per reference):
    gates[g] = x @ ih4[g] + h_0 @ hh4[g] + ib4[g] + hb4[g]   for g in I,F,G,O
    c_1 = c_0 * sigmoid(F) + sigmoid(I) * tanh(G)
    h_1 = sigmoid(O) + tanh(c_1)
Outputs: (h_1, c_1, I_g, F_g, G_g, O_g), each [B, H].

Sharding: pure data parallel over the batch axis; each of the 8 cores gets a
contiguous slab of B/8 = 16384 rows; ih/hh/ib/hb replicated. No collectives.

Per-core kernel layout (batch-major):
  - supertile = 1024 batch rows mapped as b = b0 + 8*p + r (p=partition,
    r in [0,8)); every HBM transfer is 128 descriptors x 4KB contiguous.
  - per 128-row subtile: PE-transpose x,h -> PSUM -> SBUF, then
    gates_psum[128,512] = xT.T @ Wih[128,512] + hT.T @ Whh + ones.T @ (ib+hb)
    (weights pre-concatenated over the 4 gates in the free dim; bias added
    via a rank-1 K=1 matmul; float32r streaming for 1 cycle/row).
  - ScalarE: sigmoid/tanh of the gate slices straight from PSUM;
    VectorE: raw-gate PSUM->SBUF copy and the c_1 combine;
    GpSimd: the final h_1 add.
"""

import numpy as np

import concourse.bass as bass
import concourse.bacc as bacc
import concourse.mybir as mybir
import concourse.tile as tile
from concourse import bass_utils
from concourse.masks import make_identity

N_CORES = 8
B_FULL = 131072
H = 128
ROWS_PER_CORE = B_FULL // N_CORES

SUPER = 1024          # batch rows per supertile
RPP = SUPER // 128    # rows per partition = subtiles per supertile

F32 = mybir.dt.float32
F32R = mybir.dt.float32r
AFT = mybir.ActivationFunctionType

USE_F32R = True

OUT_NAMES = ("h_1", "c_1", "I_g", "F_g", "G_g", "O_g")


def build_nc(rows=ROWS_PER_CORE, super_rows=SUPER, repeat=1, dma_only=False):
    rpp = super_rows // 128
    assert rows % super_rows == 0
    n_super = rows // super_rows

    nc = bacc.Bacc("TRN2", target_bir_lowering=False)

    x = nc.dram_tensor("x", [rows, H], F32, kind="ExternalInput")
    h0 = nc.dram_tensor("h_0", [rows, H], F32, kind="ExternalInput")
    c0 = nc.dram_tensor("c_0", [rows, H], F32, kind="ExternalInput")
    ih = nc.dram_tensor("ih", [4 * H, H], F32, kind="ExternalInput")
    hh = nc.dram_tensor("hh", [4 * H, H], F32, kind="ExternalInput")
    ib = nc.dram_tensor("ib", [4 * H], F32, kind="ExternalInput")
    hb = nc.dram_tensor("hb", [4 * H], F32, kind="ExternalInput")
    outs = {
        name: nc.dram_tensor(name, [rows, H], F32, kind="ExternalOutput")
        for name in OUT_NAMES
    }

    MMDT = F32R if USE_F32R else F32

    # HBM views: [n_super, 128 partitions, rpp*H] with b = s*super + p*rpp + r
    def view(t):
        return t.ap().rearrange("(s p r) i -> s p (r i)", p=128, r=rpp)

    xv, hv, cv = view(x), view(h0), view(c0)
    ov = {name: view(t) for name, t in outs.items()}

    with tile.TileContext(nc) as tc:
        with (
            tc.tile_pool(name="const", bufs=1) as cpool,
            tc.tile_pool(name="io", bufs=4) as iop,
            tc.tile_pool(name="trp", bufs=3, space="PSUM") as trp,
            tc.tile_pool(name="pgp", bufs=4, space="PSUM") as pgp,
            tc.tile_pool(name="sbt", bufs=3) as sbt,
            tc.tile_pool(name="gsb", bufs=3) as gsb,
            tc.tile_pool(name="actp", bufs=2) as actp,
            tc.tile_pool(name="resp", bufs=2) as resp,
        ):
            ident = cpool.tile([128, 128], F32)
            make_identity(nc, ident[:])

            # Wih[h, g*128+i] = ih[g*128+h, i]; same for Whh.
            wih_raw = cpool.tile([128, 4 * H], F32)
            whh_raw = cpool.tile([128, 4 * H], F32)
            for g in range(4):
                gs = slice(g * H, (g + 1) * H)
                nc.sync.dma_start(wih_raw[:, gs], ih.ap()[gs, :])
                nc.sync.dma_start(whh_raw[:, gs], hh.ap()[gs, :])
            # round once to the matmul streaming dtype (f32r producers must
            # explicitly round; these are one-time const-setup copies)
            wih = cpool.tile([128, 4 * H], MMDT)
            whh = cpool.tile([128, 4 * H], MMDT)
            nc.vector.tensor_copy(wih[:], wih_raw[:])
            nc.vector.tensor_copy(whh[:], whh_raw[:])

            ones_raw = cpool.tile([1, 128], F32)
            nc.vector.memset(ones_raw[:], 1.0)
            ones = cpool.tile([1, 128], MMDT)
            nc.vector.tensor_copy(ones[:], ones_raw[:])
            bib = cpool.tile([1, 4 * H], F32)
            bhb = cpool.tile([1, 4 * H], F32)
            nc.sync.dma_start(bib[:], ib.ap()[None, :])
            nc.sync.dma_start(bhb[:], hb.ap()[None, :])
            bsum_raw = cpool.tile([1, 4 * H], F32)
            nc.vector.tensor_add(bsum_raw[:], bib[:], bhb[:])
            bsum = cpool.tile([1, 4 * H], MMDT)
            nc.vector.tensor_copy(bsum[:], bsum_raw[:])

            if dma_only:
                # timing probe: identical DMA traffic, zero compute
                zg = cpool.tile([128, rpp * 512], F32)
                nc.vector.memset(zg[:], 0.0)
                for s in [s for _ in range(repeat) for s in range(n_super)]:
                    for src in (xv, hv, cv):
                        t = iop.tile([128, super_rows], F32)
                        nc.sync.dma_start(t[:], src[s])
                    nc.sync.dma_start(ov["h_1"][s], zg[:, 0 : super_rows])
                    nc.sync.dma_start(ov["c_1"][s], zg[:, 0 : super_rows])
                    zr = zg[:].rearrange("p (r gi) -> p r gi", gi=512)
                    for g, name in enumerate(("I_g", "F_g", "G_g", "O_g")):
                        nc.sync.dma_start(ov[name][s], zr[:, :, g * H : (g + 1) * H])
                nc.compile()
                return nc

            for s in [s for _ in range(repeat) for s in range(n_super)]:
                x_in = iop.tile([128, super_rows], F32)
                nc.sync.dma_start(x_in[:], xv[s])
                h_in = iop.tile([128, super_rows], F32)
                nc.sync.dma_start(h_in[:], hv[s])
                c_in = iop.tile([128, super_rows], F32)
                nc.sync.dma_start(c_in[:], cv[s])

                gates = gsb.tile([128, rpp * 512], F32)
                sigI = actp.tile([128, super_rows], F32)
                sigF = actp.tile([128, super_rows], F32)
                tanG = actp.tile([128, super_rows], F32)
                sigO = actp.tile([128, super_rows], F32)

                for r in range(rpp):
                    rs = slice(r * 128, (r + 1) * 128)
                    # both transposes into ONE psum bank as one accumulation
                    # group (disjoint slices) -> a single pair-copy to SBUF
                    tr = trp.tile([128, 512], F32)
                    nc.tensor.matmul(
                        tr[:, 0:128], x_in[:, rs], ident[:],
                        is_transpose=True, start=True, stop=False,
                    )
                    nc.tensor.matmul(
                        tr[:, 128:256], h_in[:, rs], ident[:],
                        is_transpose=True, start=False, stop=True,
                    )
                    xhT = sbt.tile([128, 256], MMDT)
                    nc.scalar.copy(xhT[:], tr[:, 0:256])

                    pg = pgp.tile([128, 512], F32)
                    nc.tensor.matmul(pg[:], xhT[:, 0:128], wih[:], start=True, stop=False)
                    nc.tensor.matmul(pg[:], xhT[:, 128:256], whh[:], start=False, stop=False)
                    nc.tensor.matmul(pg[:], ones[:], bsum[:], start=False, stop=True)

                    # raw (pre-activation) gates out
                    nc.vector.tensor_copy(gates[:, r * 512 : (r + 1) * 512], pg[:])

                # gate activations: one strided op per gate over the whole
                # supertile, reading the SBUF raw-gates copy
                gr3 = gates[:].rearrange("p (r gi) -> p r gi", gi=512)
                for g, dst in enumerate((sigI, sigF, tanG, sigO)):
                    func = AFT.Tanh if g == 2 else AFT.Sigmoid
                    src = gr3[:, :, g * 128 : (g + 1) * 128]
                    d3 = dst[:].rearrange("p (r i) -> p r i", i=128)
                    nc.scalar.activation(d3, src, func)

                t1 = resp.tile([128, super_rows], F32)
                nc.vector.tensor_mul(t1[:], c_in[:], sigF[:])
                t2 = resp.tile([128, super_rows], F32)
                nc.vector.tensor_mul(t2[:], sigI[:], tanG[:])
                c1t = resp.tile([128, super_rows], F32)
                nc.vector.tensor_add(c1t[:], t1[:], t2[:])
                th1 = actp.tile([128, super_rows], F32)
                nc.scalar.activation(th1[:], c1t[:], AFT.Tanh)
                h1t = resp.tile([128, super_rows], F32)
                nc.gpsimd.tensor_add(h1t[:], sigO[:], th1[:])

                nc.sync.dma_start(ov["h_1"][s], h1t[:])
                nc.sync.dma_start(ov["c_1"][s], c1t[:])
                gr = gates[:].rearrange("p (r gi) -> p r gi", gi=512)
                for g, name in enumerate(("I_g", "F_g", "G_g", "O_g")):
                    nc.sync.dma_start(ov[name][s], gr[:, :, g * H : (g + 1) * H])

    nc.compile()
    return nc


_NC_CACHE = {}


def _get_nc(rows=ROWS_PER_CORE):
    if rows not in _NC_CACHE:
        _NC_CACHE[rows] = build_nc(rows)
    return _NC_CACHE[rows]


def run_sharded(x, h_0, c_0, ih, hh, ib, hb, **spmd_kwargs):
    x = np.asarray(x, dtype=np.float32)
    h_0 = np.asarray(h_0, dtype=np.float32)
    c_0 = np.asarray(c_0, dtype=np.float32)
    ih = np.ascontiguousarray(np.asarray(ih, dtype=np.float32))
    hh = np.ascontiguousarray(np.asarray(hh, dtype=np.float32))
    ib = np.ascontiguousarray(np.asarray(ib, dtype=np.float32))
    hb = np.ascontiguousarray(np.asarray(hb, dtype=np.float32))

    nc = _get_nc()
    in_maps = []
    for i in range(N_CORES):
        sl = slice(i * ROWS_PER_CORE, (i + 1) * ROWS_PER_CORE)
        in_maps.append(
            dict(
                x=np.ascontiguousarray(x[sl]),
                h_0=np.ascontiguousarray(h_0[sl]),
                c_0=np.ascontiguousarray(c_0[sl]),
                ih=ih,
                hh=hh,
                ib=ib,
                hb=hb,
            )
        )
    res = bass_utils.run_bass_kernel_spmd(
        nc, in_maps, core_ids=list(range(N_CORES)), **spmd_kwargs
    )
    outs = res.results
    full = tuple(
        np.concatenate([outs[i][name] for i in range(N_CORES)], axis=0)
        for name in OUT_NAMES
    )
    return full, res


def kernel(x, h_0, c_0, ih, hh, ib, hb):
    full, _ = run_sharded(x, h_0, c_0, ih, hh, ib, hb)
    return full
